# revision 4
# baseline (speedup 1.0000x reference)
"""Trainium2 Bass kernel for nn_MatchSegmentation.

matching[k] = argmin_g ce[k,g], ce = mean_n BCE(segmentation[n,k], gt[g,n]).
Since B[k] = sum_n log(1-s+eps) is constant per k and -1/n is a negative
scale, argmin_g ce[k,:] == argmin_g (C-A)[k,:] with
  A = g @ log(s+eps)^T partials,  C = g @ log(1-s+eps)^T partials.

Sharding: pixels split 8 ways (8192/core). Per core:
  - seg is host-quantized to uint16 (u = round(s*65536); the <=2^-17
    quantization error is ~40x below the argmin safety margin) and
    host-swizzled so partition p holds pixels {c*128+p}: seg[p, c*K+k].
  - DMA (4 blocks, HWDGE sync queue) -> SBUF
  - ACT computes log(u*2^-16 + eps) and log(-u*2^-16 + 1+eps) per block
    into a concatenated (128, nch, 2K) bf16 tile (scalar engine Ln, the
    free input affine gives both logs from the same uint16 data)
  - PE accumulates psAC[g, 0:K] += gt_c^T @ log_s, psAC[g, K:2K] += ..log_1ms
    (one 256-wide bf16 matmul per 128-pixel chunk, fp32 PSUM)
  - epilogue: PSUM -> SBUF copy, DMA out the (22, 256) A|C partial sums.
Host: sum the 8 partials, D = C-A, mask padded g slots, argmin -> (K,1).

The engine-time budget per core: ACT 2 passes = 16384 cycles @1.2GHz =
13.7us (the bound), DMA 2.4MB @358GB/s = 6.8us, PE 16384 moving columns
@2.4GHz = 7us; DMA and PE hide under ACT.
"""

import numpy as np
import ml_dtypes
from contextlib import ExitStack

import concourse.bass as bass
import concourse.tile as tile
from concourse import bacc, mybir
from concourse.bass_utils import run_bass_kernel_spmd

F32 = mybir.dt.float32
BF16 = mybir.dt.bfloat16
U16 = mybir.dt.uint16

NCORES = 8
N_FULL = 65536          # h*w pixels
K = 128                 # segmentation channels
GMAX = 21               # gt instances provided
GP = 22                 # padded instance slots (col 21 always padding)
NSHARD = N_FULL // NCORES   # 8192 pixels per core
CHUNK = 128             # pixels per matmul (contraction = partition dim)
NCHUNK = NSHARD // CHUNK    # 64
EPS = 1e-6

# DMA blocks (chunks per seg dma_start) and ACT blocks (chunks per Ln
# instruction pair). ACT blocks must nest inside DMA blocks. Small first
# block -> early ACT start; small last block -> short matmul tail.
DMA_BLOCKS = [4, 12, 24, 24]
ACT_BLOCKS = [4, 12, 24, 20, 4]
assert sum(DMA_BLOCKS) == NCHUNK and sum(ACT_BLOCKS) == NCHUNK
N_WARM_MM = 12          # dummy matmuls to pull the PE HAM clock-gate open

_PROG = {}
MODE = "devlog"         # "devlog": logs on device; "hostlog": logs on host


def _build_program(mode):
    nc = bacc.Bacc(
        "TRN2",
        target_bir_lowering=False,
        debug=False,
        enable_asserts=False,
        num_devices=NCORES,
    )

    devlog = mode == "devlog"
    if devlog:
        seg_d = nc.dram_tensor("seg", [128, NCHUNK * K], U16, kind="ExternalInput")
        bias_d = nc.dram_tensor("bias2", [128, 2], F32, kind="ExternalInput")
        out_w = 2 * K
    else:
        seg_d = nc.dram_tensor("seg", [128, NCHUNK * K], BF16, kind="ExternalInput")
        out_w = K
    gt_d = nc.dram_tensor("gt", [128, NCHUNK * GP], BF16, kind="ExternalInput")
    out_d = nc.dram_tensor("out", [GP, out_w], F32, kind="ExternalOutput")

    with tile.TileContext(nc) as tc, ExitStack() as ctx:
        segp = ctx.enter_context(tc.tile_pool(name="segp", bufs=1))
        logp = ctx.enter_context(tc.tile_pool(name="logp", bufs=1))
        gtp = ctx.enter_context(tc.tile_pool(name="gtp", bufs=1))
        psp = ctx.enter_context(tc.tile_pool(name="psp", bufs=1, space="PSUM"))
        sml = ctx.enter_context(tc.tile_pool(name="sml", bufs=1))

        seg_ap = seg_d.ap()
        gt_ap = gt_d.ap()

        # --- t=0 prefetches and warmups, all on distinct queues ---
        # seg blocks stream on the sync HWDGE queue. Keep every DMA a flat
        # [128, n] contiguous-per-partition transfer: multi-dim inner APs
        # make the DGE emit one descriptor per inner row, and the resulting
        # descriptor storm clogs the 16 shared SDMA engines for microseconds.
        seg_tiles = []
        off = 0
        for b, nch in enumerate(DMA_BLOCKS):
            st = segp.tile([128, nch, K], seg_d.dtype, name=f"seg{b}", tag=f"seg{b}")
            nc.sync.dma_start(
                st[:].rearrange("p c k -> p (c k)"),
                seg_ap[:, off * K : (off + nch) * K],
            )
            seg_tiles.append((off, st))
            off += nch

        # gt + bias prefetch on the gpsimd SWDGE queue (kept off sync/scalar)
        gt_t = gtp.tile([128, NCHUNK * GP], BF16)
        nc.gpsimd.dma_start(gt_t[:], gt_ap)
        if devlog:
            bias_t = sml.tile([128, 2], F32)
            nc.gpsimd.dma_start(bias_t[:], bias_d.ap())

            # Warm the ACT Ln table at t=0 (1.3us load hides under DMA).
            dummy = sml.tile([1, 8], F32)
            nc.vector.memset(dummy[:], 1.0)
            nc.scalar.activation(dummy[:], dummy[:], mybir.ActivationFunctionType.Ln)

        # PE HAM warmup: back-to-back dummy matmuls into a scratch PSUM bank
        # while the first seg block is still in flight.
        if N_WARM_MM:
            wl = sml.tile([128, GP], BF16)
            wr = sml.tile([128, K], BF16)
            nc.vector.memset(wl[:], 0.0)
            nc.vector.memset(wr[:], 0.0)
            ps_w = psp.tile([GP, K], F32, name="ps_warm", tag="ps_warm")
            for i in range(N_WARM_MM):
                nc.tensor.matmul(ps_w[:], lhsT=wl[:], rhs=wr[:], start=True, stop=True)

        # --- main pipeline: ACT (2 Ln passes per block) + PE accumulate ---
        psAC = psp.tile([GP, out_w], F32, name="psAC", tag="psAC")

        def seg_slice(off, nch):
            """View of chunks [off, off+nch) inside its DMA-block tile."""
            for boff, st in seg_tiles:
                if boff <= off and off + nch <= boff + st.shape[1]:
                    return st[:, off - boff : off - boff + nch, :]
            raise AssertionError("ACT block not nested in a DMA block")

        gc = 0
        for a, nch in enumerate(ACT_BLOCKS):
            if devlog:
                lt = logp.tile([128, nch, 2 * K], BF16, name=f"log{a}", tag=f"log{a}")
                ss = seg_slice(gc, nch)
                nc.scalar.activation(
                    lt[:, :, 0:K], ss,
                    mybir.ActivationFunctionType.Ln,
                    bias=bias_t[:, 0:1], scale=1.0 / 65536.0,
                )
                nc.scalar.activation(
                    lt[:, :, K : 2 * K], ss,
                    mybir.ActivationFunctionType.Ln,
                    bias=bias_t[:, 1:2], scale=-1.0 / 65536.0,
                )
            else:
                lt = seg_slice(gc, nch)

            for c in range(nch):
                nc.tensor.matmul(
                    psAC[:],
                    lhsT=gt_t[:, (gc + c) * GP : (gc + c + 1) * GP],
                    rhs=lt[:, c, :],
                    start=(gc + c == 0),
                    stop=(gc + c == NCHUNK - 1),
                )
            gc += nch

        # --- epilogue: PSUM -> SBUF -> HBM; host reduces across cores ---
        ac_sb = sml.tile([GP, out_w], F32)
        nc.vector.tensor_copy(ac_sb[:], psAC[:])
        nc.sync.dma_start(out_d.ap(), ac_sb[:])

    nc.compile()
    return nc


def _prepare_in_maps(segmentation, gt_instance, mode):
    seg = np.asarray(segmentation, dtype=np.float32)
    assert seg.shape == (N_FULL, K)
    if mode == "devlog":
        seg = np.clip(np.rint(seg * 65536.0), 0.0, 65535.0).astype(np.uint16)
    else:
        d = np.log(1.0 - seg + EPS) - np.log(seg + EPS)
        seg = d.astype(ml_dtypes.bfloat16)
    gt = np.asarray(gt_instance)
    gmax = gt.shape[0]

    gpad = np.zeros((N_FULL, GP), dtype=np.float32)
    gpad[:, :gmax] = gt.reshape(gmax, -1).T
    gpad = gpad.astype(ml_dtypes.bfloat16)

    bias2 = np.empty((128, 2), dtype=np.float32)
    bias2[:, 0] = EPS
    bias2[:, 1] = 1.0 + EPS

    in_maps = []
    for c in range(NCORES):
        lo = c * NSHARD
        gt_core = (
            gpad[lo : lo + NSHARD]
            .reshape(NCHUNK, CHUNK, GP)
            .transpose(1, 0, 2)
            .reshape(CHUNK, NCHUNK * GP)
        )
        seg_core = (
            seg[lo : lo + NSHARD]
            .reshape(NCHUNK, CHUNK, K)
            .transpose(1, 0, 2)
            .reshape(CHUNK, NCHUNK * K)
        )
        m = {
            "seg": np.ascontiguousarray(seg_core),
            "gt": np.ascontiguousarray(gt_core),
        }
        if mode == "devlog":
            m["bias2"] = bias2
        in_maps.append(m)
    return in_maps


LAST_RESULTS = None


def run(inputs, trace=False, mode=None, **kwargs):
    global LAST_RESULTS
    mode = mode or MODE
    if mode not in _PROG:
        _PROG[mode] = _build_program(mode)
    in_maps = _prepare_in_maps(inputs["segmentation"], inputs["gt_instance"], mode)
    res = run_bass_kernel_spmd(
        _PROG[mode], in_maps, core_ids=list(range(NCORES)), trace=trace, **kwargs
    )
    LAST_RESULTS = res
    gpn = int(inputs["gt_plane_num"])
    acc = np.sum([np.asarray(r["out"], np.float64) for r in res.results], axis=0)
    if mode == "devlog":
        d = acc[:, K : 2 * K] - acc[:, 0:K]   # C - A, (GP, K)
    else:
        d = acc                               # already sum g*(log1ms-logs)
    d[min(gpn, GP):, :] = np.inf
    return d.argmin(axis=0).astype(np.int32).reshape(K, 1)


def kernel(**inputs):
    return run(inputs)


# revision 5
# speedup vs baseline: 1.1630x; 1.1630x over previous
"""Trainium2 Bass kernel for nn_MatchSegmentation.

matching[k] = argmin_g ce[k,g], ce = mean_n BCE(segmentation[n,k], gt[g,n]).
Since B[k] = sum_n log(1-s+eps) is constant per k and -1/n is a negative
scale, argmin_g ce[k,:] == argmin_g (C-A)[k,:] with
  A[k,g] = sum_n log(s+eps)[k,n] g[g,n],  C likewise with log(1-s+eps).

Sharding: pixels split 8 ways (8192/core). Per core:
  - seg is host-quantized to uint16 (u = round(s*65536); the <=2^-17
    quantization error is ~40x below the argmin safety margin) and
    host-swizzled so partition p holds pixels {c*128+p}: seg[p, c*K+k].
  - seg blocks stream on BOTH the sync HWDGE and gpsimd SWDGE rings
    (alternating) so the per-DMA completion receipts overlap; every DMA
    is a flat [128, n] contiguous-per-partition transfer (multi-dim inner
    APs degenerate into per-row descriptor storms).
  - ACT (scalar engine Ln) computes log(u*2^-16 + eps) and
    log(-u*2^-16 + 1+eps) per block into a (128, nch, 2K) bf16 tile;
    2 passes x 8192 cols = 13.7us @1.2GHz is the kernel's engine bound.
  - PE accumulates psAC[g, :K] += gt_c^T @ log_s | psAC[g, K:] += log_1ms
    (one 256-wide bf16 matmul per 128-pixel chunk, fp32 PSUM); a dummy
    matmul burst at t=0 pulls the PE HAM clock-gate open.
  - epilogue: PSUM -> SBUF copy, DMA out the (22, 256) A|C partial sums.
Host: sum the 8 partials, D = C-A, mask padded g slots, argmin -> (K,1).
"""

import numpy as np
import ml_dtypes
from contextlib import ExitStack

import concourse.bass as bass
import concourse.tile as tile
from concourse import bacc, mybir
from concourse.bass_utils import run_bass_kernel_spmd

F32 = mybir.dt.float32
BF16 = mybir.dt.bfloat16
U16 = mybir.dt.uint16

NCORES = 8
N_FULL = 65536          # h*w pixels
K = 128                 # segmentation channels
GMAX = 21               # gt instances provided
GP = 22                 # padded instance slots (col 21 always padding)
NSHARD = N_FULL // NCORES   # 8192 pixels per core
CHUNK = 128             # pixels per matmul (contraction = partition dim)
NCHUNK = NSHARD // CHUNK    # 64
EPS = 1e-6

# seg DMA blocks: (chunks, engine). Alternating rings overlaps the ~1.3us
# per-DMA completion receipts; sizes grow so delivery stays ahead of the
# 213ns/chunk ACT consumption.
DMA_BLOCKS = [(4, "sync"), (8, "gpsimd"), (20, "sync"), (32, "gpsimd")]
# ACT blocks (chunks per Ln instruction pair), nested inside DMA blocks.
ACT_BLOCKS = [4, 8, 20, 28, 4]
assert sum(n for n, _ in DMA_BLOCKS) == NCHUNK and sum(ACT_BLOCKS) == NCHUNK
N_WARM_MM = 12          # dummy matmuls to pull the PE HAM clock-gate open

_PROG = {}
MODE = "devlog"         # "devlog": logs on device; "hostlog": logs on host


def _build_program(mode):
    nc = bacc.Bacc(
        "TRN2",
        target_bir_lowering=False,
        debug=False,
        enable_asserts=False,
        num_devices=NCORES,
    )

    devlog = mode == "devlog"
    seg_dt = U16 if devlog else BF16
    out_w = 2 * K if devlog else K
    seg_d = nc.dram_tensor("seg", [128, NCHUNK * K], seg_dt, kind="ExternalInput")
    gt_d = nc.dram_tensor("gt", [128, NCHUNK * GP], BF16, kind="ExternalInput")
    out_d = nc.dram_tensor("out", [GP, out_w], F32, kind="ExternalOutput")

    with tile.TileContext(nc) as tc, ExitStack() as ctx:
        segp = ctx.enter_context(tc.tile_pool(name="segp", bufs=1))
        logp = ctx.enter_context(tc.tile_pool(name="logp", bufs=1))
        gtp = ctx.enter_context(tc.tile_pool(name="gtp", bufs=1))
        psp = ctx.enter_context(tc.tile_pool(name="psp", bufs=1, space="PSUM"))
        sml = ctx.enter_context(tc.tile_pool(name="sml", bufs=1))

        seg_ap = seg_d.ap()

        # --- t=0: seg blocks on two rings; gt queued behind sync seg so it
        # stays off the first blocks' completion path (MMs only need it
        # after the first ACT pair anyway).
        seg_tiles = []
        off = 0
        for b, (nch, eng) in enumerate(DMA_BLOCKS):
            st = segp.tile([128, nch, K], seg_dt, name=f"seg{b}", tag=f"seg{b}")
            dma = nc.sync.dma_start if eng == "sync" else nc.gpsimd.dma_start
            dma(
                st[:].rearrange("p c k -> p (c k)"),
                seg_ap[:, off * K : (off + nch) * K],
            )
            seg_tiles.append((off, st))
            off += nch

        gt_t = gtp.tile([128, NCHUNK * GP], BF16)
        nc.sync.dma_start(gt_t[:], gt_d.ap())

        if devlog:
            # ACT affine constants built on-device (no DMA needed).
            bias_t = sml.tile([128, 2], F32)
            nc.vector.memset(bias_t[:, 0:1], EPS)
            nc.vector.memset(bias_t[:, 1:2], 1.0 + EPS)

            # Warm the ACT Ln table at t=0 (1.3us load hides under DMA).
            dummy = sml.tile([1, 8], F32)
            nc.vector.memset(dummy[:], 1.0)
            nc.scalar.activation(dummy[:], dummy[:], mybir.ActivationFunctionType.Ln)

        # PE HAM warmup: back-to-back dummy matmuls into a scratch PSUM bank
        # while the first seg block is still in flight.
        if N_WARM_MM:
            wl = sml.tile([128, GP], BF16)
            wr = sml.tile([128, K], BF16)
            nc.vector.memset(wl[:], 0.0)
            nc.vector.memset(wr[:], 0.0)
            ps_w = psp.tile([GP, K], F32, name="ps_warm", tag="ps_warm")
            for i in range(N_WARM_MM):
                nc.tensor.matmul(ps_w[:], lhsT=wl[:], rhs=wr[:], start=True, stop=True)

        # --- main pipeline: ACT (2 Ln passes per block) + PE accumulate ---
        psAC = psp.tile([GP, out_w], F32, name="psAC", tag="psAC")

        def seg_slice(off, nch):
            """View of chunks [off, off+nch) inside their DMA-block tile."""
            for boff, st in seg_tiles:
                if boff <= off and off + nch <= boff + st.shape[1]:
                    return st[:, off - boff : off - boff + nch, :]
            raise AssertionError("ACT block not nested in a DMA block")

        gc = 0
        for a, nch in enumerate(ACT_BLOCKS):
            if devlog:
                lt = logp.tile([128, nch, 2 * K], BF16, name=f"log{a}", tag=f"log{a}")
                ss = seg_slice(gc, nch)
                nc.scalar.activation(
                    lt[:, :, 0:K], ss,
                    mybir.ActivationFunctionType.Ln,
                    bias=bias_t[:, 0:1], scale=1.0 / 65536.0,
                )
                nc.scalar.activation(
                    lt[:, :, K : 2 * K], ss,
                    mybir.ActivationFunctionType.Ln,
                    bias=bias_t[:, 1:2], scale=-1.0 / 65536.0,
                )
            else:
                lt = seg_slice(gc, nch)

            for c in range(nch):
                nc.tensor.matmul(
                    psAC[:],
                    lhsT=gt_t[:, (gc + c) * GP : (gc + c + 1) * GP],
                    rhs=lt[:, c, :],
                    start=(gc + c == 0),
                    stop=(gc + c == NCHUNK - 1),
                )
            gc += nch

        # --- epilogue: PSUM -> SBUF -> HBM; host reduces across cores ---
        ac_sb = sml.tile([GP, out_w], F32)
        nc.vector.tensor_copy(ac_sb[:], psAC[:])
        nc.sync.dma_start(out_d.ap(), ac_sb[:])

    nc.compile()
    return nc


def _prepare_in_maps(segmentation, gt_instance, mode):
    seg = np.asarray(segmentation, dtype=np.float32)
    assert seg.shape == (N_FULL, K)
    if mode == "devlog":
        seg = np.clip(np.rint(seg * 65536.0), 0.0, 65535.0).astype(np.uint16)
    else:
        d = np.log(1.0 - seg + EPS) - np.log(seg + EPS)
        seg = d.astype(ml_dtypes.bfloat16)
    gt = np.asarray(gt_instance)
    gmax = gt.shape[0]

    gpad = np.zeros((N_FULL, GP), dtype=np.float32)
    gpad[:, :gmax] = gt.reshape(gmax, -1).T
    gpad = gpad.astype(ml_dtypes.bfloat16)

    in_maps = []
    for c in range(NCORES):
        lo = c * NSHARD
        gt_core = (
            gpad[lo : lo + NSHARD]
            .reshape(NCHUNK, CHUNK, GP)
            .transpose(1, 0, 2)
            .reshape(CHUNK, NCHUNK * GP)
        )
        seg_core = (
            seg[lo : lo + NSHARD]
            .reshape(NCHUNK, CHUNK, K)
            .transpose(1, 0, 2)
            .reshape(CHUNK, NCHUNK * K)
        )
        in_maps.append({
            "seg": np.ascontiguousarray(seg_core),
            "gt": np.ascontiguousarray(gt_core),
        })
    return in_maps


LAST_RESULTS = None


def run(inputs, trace=False, mode=None, **kwargs):
    global LAST_RESULTS
    mode = mode or MODE
    if mode not in _PROG:
        _PROG[mode] = _build_program(mode)
    in_maps = _prepare_in_maps(inputs["segmentation"], inputs["gt_instance"], mode)
    res = run_bass_kernel_spmd(
        _PROG[mode], in_maps, core_ids=list(range(NCORES)), trace=trace, **kwargs
    )
    LAST_RESULTS = res
    gpn = int(inputs["gt_plane_num"])
    acc = np.sum([np.asarray(r["out"], np.float64) for r in res.results], axis=0)
    if mode == "devlog":
        d = acc[:, K : 2 * K] - acc[:, 0:K]   # C - A, (GP, K)
    else:
        d = acc                               # already sum g*(log1ms-logs)
    d[min(gpn, GP):, :] = np.inf
    return d.argmin(axis=0).astype(np.int32).reshape(K, 1)


def kernel(**inputs):
    return run(inputs)


# revision 8
# speedup vs baseline: 1.6410x; 1.4109x over previous
"""Trainium2 Bass kernel for nn_MatchSegmentation.

matching[k] = argmin_g ce[k,g], ce = mean_n BCE(segmentation[n,k], gt[g,n]).
Since B[k] = sum_n log(1-s+eps) is constant per k and -1/n is a negative
scale, argmin_g ce[k,:] == argmin_g (C-A)[k,:] with
  A[k,g] = sum_n log(s+eps)[k,n] g[g,n],  C likewise with log(1-s+eps).

Sharding: pixels split 8 ways (8192/core). Per core:
  - seg is host-quantized to uint16 (u = round(s*65536); the <=2^-17
    quantization error is ~40x below the argmin safety margin) and
    host-swizzled so partition p holds pixels {c*128+p}: seg[p, c*K+k].
  - seg blocks stream on BOTH the sync HWDGE and gpsimd SWDGE rings
    (alternating) so the per-DMA completion receipts overlap; every DMA
    is a flat [128, n] contiguous-per-partition transfer (multi-dim inner
    APs degenerate into per-row descriptor storms).
  - ACT (scalar engine Ln) computes log(u*2^-16 + eps) and
    log(-u*2^-16 + 1+eps) per block into a (128, nch, 2K) bf16 tile;
    2 passes x 8192 cols = 13.7us @1.2GHz is the kernel's engine bound.
  - PE accumulates psAC[g, :K] += gt_c^T @ log_s | psAC[g, K:] += log_1ms
    (one 256-wide bf16 matmul per 128-pixel chunk, fp32 PSUM); a dummy
    matmul burst at t=0 pulls the PE HAM clock-gate open.
  - epilogue: PSUM -> SBUF copy, DMA out the (22, 256) A|C partial sums.
Host: sum the 8 partials, D = C-A, mask padded g slots, argmin -> (K,1).
"""

import numpy as np
import ml_dtypes
from contextlib import ExitStack

import concourse.bass as bass
import concourse.tile as tile
from concourse import bacc, mybir
from concourse.bass_utils import run_bass_kernel_spmd

F32 = mybir.dt.float32
BF16 = mybir.dt.bfloat16
U16 = mybir.dt.uint16

NCORES = 8
N_FULL = 65536          # h*w pixels
K = 128                 # segmentation channels
GMAX = 21               # gt instances provided
GP = 22                 # padded instance slots (col 21 always padding)
NSHARD = N_FULL // NCORES   # 8192 pixels per core
CHUNK = 128             # pixels per matmul (contraction = partition dim)
NCHUNK = NSHARD // CHUNK    # 64
EPS = 1e-6

# seg DMA blocks: (chunks, engine). Alternating rings overlaps the ~1.3us
# per-DMA completion receipts; sizes grow so delivery stays ahead of the
# 213ns/chunk ACT consumption.
DMA_BLOCKS = [(4, "sync"), (8, "gpsimd"), (20, "sync"), (32, "gpsimd")]
# ACT blocks (chunks per Ln instruction pair), nested inside DMA blocks.
# Matmuls for a block only start once both its Ln passes finish, so blocks
# taper at the end to keep the post-ACT matmul tail short.
ACT_BLOCKS = [4, 8, 20, 16, 10, 4, 2]
assert sum(n for n, _ in DMA_BLOCKS) == NCHUNK and sum(ACT_BLOCKS) == NCHUNK
N_WARM_MM = 12          # dummy matmuls to pull the PE HAM clock-gate open

_PROG = {}
MODE = "devlog"         # "devlog": logs on device; "hostlog": logs on host


def _build_program(mode):
    nc = bacc.Bacc(
        "TRN2",
        target_bir_lowering=False,
        debug=False,
        enable_asserts=False,
        num_devices=NCORES,
    )

    devlog = mode == "devlog"
    seg_dt = U16 if devlog else BF16
    out_w = 2 * K if devlog else K
    seg_d = nc.dram_tensor("seg", [128, NCHUNK * K], seg_dt, kind="ExternalInput")
    gt_d = nc.dram_tensor("gt", [128, NCHUNK * GP], BF16, kind="ExternalInput")
    out_d = nc.dram_tensor("out", [GP, out_w], F32, kind="ExternalOutput")

    with tile.TileContext(nc) as tc, ExitStack() as ctx:
        segp = ctx.enter_context(tc.tile_pool(name="segp", bufs=1))
        logp = ctx.enter_context(tc.tile_pool(name="logp", bufs=1))
        gtp = ctx.enter_context(tc.tile_pool(name="gtp", bufs=1))
        psp = ctx.enter_context(tc.tile_pool(name="psp", bufs=1, space="PSUM"))
        sml = ctx.enter_context(tc.tile_pool(name="sml", bufs=1))

        seg_ap = seg_d.ap()

        # --- t=0: gt first on the gpsimd ring (matmuls need it right after
        # the first ACT pair), seg blocks alternating on both rings.
        gt_t = gtp.tile([128, NCHUNK * GP], BF16)
        nc.gpsimd.dma_start(gt_t[:], gt_d.ap())

        seg_tiles = []
        off = 0
        for b, (nch, eng) in enumerate(DMA_BLOCKS):
            st = segp.tile([128, nch, K], seg_dt, name=f"seg{b}", tag=f"seg{b}")
            dma = nc.sync.dma_start if eng == "sync" else nc.gpsimd.dma_start
            dma(
                st[:].rearrange("p c k -> p (c k)"),
                seg_ap[:, off * K : (off + nch) * K],
            )
            seg_tiles.append((off, st))
            off += nch

        if devlog:
            # ACT affine constants built on-device (no DMA needed).
            bias_t = sml.tile([128, 2], F32)
            nc.vector.memset(bias_t[:, 0:1], EPS)
            nc.vector.memset(bias_t[:, 1:2], 1.0 + EPS)

            # Warm the ACT Ln table at t=0 (1.3us load hides under DMA).
            dummy = sml.tile([1, 8], F32)
            nc.vector.memset(dummy[:], 1.0)
            nc.scalar.activation(dummy[:], dummy[:], mybir.ActivationFunctionType.Ln)

        # PE HAM warmup: back-to-back dummy matmuls into a scratch PSUM bank
        # while the first seg block is still in flight.
        if N_WARM_MM:
            wl = sml.tile([128, GP], BF16)
            wr = sml.tile([128, K], BF16)
            nc.vector.memset(wl[:], 0.0)
            nc.vector.memset(wr[:], 0.0)
            ps_w = psp.tile([GP, K], F32, name="ps_warm", tag="ps_warm")
            for i in range(N_WARM_MM):
                nc.tensor.matmul(ps_w[:], lhsT=wl[:], rhs=wr[:], start=True, stop=True)

        # --- main pipeline: ACT (2 Ln passes per block) + PE accumulate ---
        psAC = psp.tile([GP, out_w], F32, name="psAC", tag="psAC")

        def seg_slice(off, nch):
            """View of chunks [off, off+nch) inside their DMA-block tile."""
            for boff, st in seg_tiles:
                if boff <= off and off + nch <= boff + st.shape[1]:
                    return st[:, off - boff : off - boff + nch, :]
            raise AssertionError("ACT block not nested in a DMA block")

        gc = 0
        for a, nch in enumerate(ACT_BLOCKS):
            if devlog:
                lt = logp.tile([128, nch, 2 * K], BF16, name=f"log{a}", tag=f"log{a}")
                ss = seg_slice(gc, nch)
                nc.scalar.activation(
                    lt[:, :, 0:K], ss,
                    mybir.ActivationFunctionType.Ln,
                    bias=bias_t[:, 0:1], scale=1.0 / 65536.0,
                )
                nc.scalar.activation(
                    lt[:, :, K : 2 * K], ss,
                    mybir.ActivationFunctionType.Ln,
                    bias=bias_t[:, 1:2], scale=-1.0 / 65536.0,
                )
            else:
                lt = seg_slice(gc, nch)

            for c in range(nch):
                nc.tensor.matmul(
                    psAC[:],
                    lhsT=gt_t[:, (gc + c) * GP : (gc + c + 1) * GP],
                    rhs=lt[:, c, :],
                    start=(gc + c == 0),
                    stop=(gc + c == NCHUNK - 1),
                )
            gc += nch

        # --- epilogue: PSUM -> SBUF -> HBM; host reduces across cores ---
        ac_sb = sml.tile([GP, out_w], F32)
        nc.vector.tensor_copy(ac_sb[:], psAC[:])
        nc.sync.dma_start(out_d.ap(), ac_sb[:])

    nc.compile()
    return nc


def _prepare_in_maps(segmentation, gt_instance, mode):
    seg = np.asarray(segmentation, dtype=np.float32)
    assert seg.shape == (N_FULL, K)
    if mode == "devlog":
        seg = np.clip(np.rint(seg * 65536.0), 0.0, 65535.0).astype(np.uint16)
    else:
        d = np.log(1.0 - seg + EPS) - np.log(seg + EPS)
        seg = d.astype(ml_dtypes.bfloat16)
    gt = np.asarray(gt_instance)
    gmax = gt.shape[0]

    gpad = np.zeros((N_FULL, GP), dtype=np.float32)
    gpad[:, :gmax] = gt.reshape(gmax, -1).T
    gpad = gpad.astype(ml_dtypes.bfloat16)

    in_maps = []
    for c in range(NCORES):
        lo = c * NSHARD
        gt_core = (
            gpad[lo : lo + NSHARD]
            .reshape(NCHUNK, CHUNK, GP)
            .transpose(1, 0, 2)
            .reshape(CHUNK, NCHUNK * GP)
        )
        seg_core = (
            seg[lo : lo + NSHARD]
            .reshape(NCHUNK, CHUNK, K)
            .transpose(1, 0, 2)
            .reshape(CHUNK, NCHUNK * K)
        )
        in_maps.append({
            "seg": np.ascontiguousarray(seg_core),
            "gt": np.ascontiguousarray(gt_core),
        })
    return in_maps


LAST_RESULTS = None


def run(inputs, trace=False, mode=None, **kwargs):
    global LAST_RESULTS
    mode = mode or MODE
    if mode not in _PROG:
        _PROG[mode] = _build_program(mode)
    in_maps = _prepare_in_maps(inputs["segmentation"], inputs["gt_instance"], mode)
    res = run_bass_kernel_spmd(
        _PROG[mode], in_maps, core_ids=list(range(NCORES)), trace=trace, **kwargs
    )
    LAST_RESULTS = res
    gpn = int(inputs["gt_plane_num"])
    acc = np.sum([np.asarray(r["out"], np.float64) for r in res.results], axis=0)
    if mode == "devlog":
        d = acc[:, K : 2 * K] - acc[:, 0:K]   # C - A, (GP, K)
    else:
        d = acc                               # already sum g*(log1ms-logs)
    d[min(gpn, GP):, :] = np.inf
    return d.argmin(axis=0).astype(np.int32).reshape(K, 1)


def kernel(**inputs):
    return run(inputs)
